# revision 13
# baseline (speedup 1.0000x reference)
"""Trainium2 Bass kernel for nn_MultiHeadAttention_89318139888179.

Problem: B=4, S=2048, D=1024, H=16 heads (hd=64) fp32 multi-head attention
with (quirky) RoPE, y = softmax((rot(q) @ rot(k)^T)/8) v, projections are
x @ W^T + b with W [e,d].

Sharding: 8 cores = 4 batches x 2 query-halves. Each core computes K/V for
its whole batch (2048 keys) and attention for its 1024 queries, producing a
disjoint [1024, 1024] slice of the output. No collectives.

Layout strategy (per core):
 - All device tensors pre-transposed on host so every matmul contraction dim
   sits on SBUF partitions. Host also interleaves Wq/Wk output rows so the
   RoPE rotation pairs sit on adjacent (even,odd) partitions, making the
   rotation's partner-swap a DVE stream_shuffle (32-lane even/odd swap).
 - Projections produce Q^T/K^T as [e', s] tiles (e' on partitions), V as
   [s, e] tiles -- both directly consumable by the attention matmuls.
 - Scores are computed transposed, scoresT[k, q] = K^T.T @ Q^T, exp'd on the
   scalar engine (scale=1/8 fused, no max subtraction: |scores| < ~6), and
   fed straight into PV: ctx^T[hd, q] = V_aug.T @ expT with a ones column
   appended to V so row 64 of the PV accumulator is the softmax denominator.
 - 1/Z via exp(-ln(Z)) on the scalar engine (same ACT table set as exp),
   broadcast across partitions with a K=1 matmul, normalize on DVE.
 - Out-projection contracts ctx^T tiles against Wo^T tiles, output lands
   [s, e] and DMAs straight out.

dtypes: bf16 matmul inputs everywhere (PSUM accumulation fp32), fp32 output.
bv/bo are folded into the output on the host (softmax rows sum to 1, so bv
contributes exactly Wo @ bv); bq/bk are added on device via K=1 matmuls.
"""

import os
import numpy as np
import ml_dtypes
from contextlib import ExitStack

import concourse.bacc as bacc
import concourse.bass as bass
import concourse.tile as tile
import concourse.mybir as mybir
from concourse.bass_utils import run_bass_kernel_spmd

BF16 = mybir.dt.bfloat16
F32 = mybir.dt.float32
F32R = mybir.dt.float32r

B, S, D, H = 4, 2048, 1024, 16
HD = 64
NCORE = 8
SQ = S // 2  # queries per core
NP_BF16 = ml_dtypes.bfloat16

# True: rely on engines accepting different in/out base partitions
# (packed 128-row ctx tiles, K=128 out-projection).
PACKED = True

_EO_MASK = [x for i in range(16) for x in (2 * i + 1, 2 * i)]


def _build_kernel(with_bias=True):
    nc = bacc.Bacc("TRN2", target_bir_lowering=False, debug=False,
                   num_devices=NCORE)

    xt_d = nc.dram_tensor("xt", [D, S], BF16, kind="ExternalInput")
    wqt_d = nc.dram_tensor("wqt", [D, D], BF16, kind="ExternalInput")
    wkt_d = nc.dram_tensor("wkt", [D, D], BF16, kind="ExternalInput")
    wvt_d = nc.dram_tensor("wvt", [D, D], BF16, kind="ExternalInput")
    if PACKED:
        wot_d = nc.dram_tensor("wot", [8, 128, D], BF16, kind="ExternalInput")
    else:
        wot_d = nc.dram_tensor("wot", [H, HD, D], BF16, kind="ExternalInput")
    bq_d = nc.dram_tensor("bq", [1, D], BF16, kind="ExternalInput")
    bk_d = nc.dram_tensor("bk", [1, D], BF16, kind="ExternalInput")
    cc_d = nc.dram_tensor("cc", [128, S], BF16, kind="ExternalInput")
    sg_d = nc.dram_tensor("sg", [128, S], BF16, kind="ExternalInput")
    out_d = nc.dram_tensor("out", [SQ, D], F32, kind="ExternalOutput")

    # ctx^T spill buffer
    if PACKED:
        ctx_d = nc.dram_tensor("ctxs", [8, 128, SQ], BF16, kind="Internal")
    else:
        ctx_d = nc.dram_tensor("ctxs", [H, HD, SQ], BF16, kind="Internal")

    with tile.TileContext(nc) as tc, ExitStack() as ex:
        const_p = ex.enter_context(tc.tile_pool(name="const", bufs=1))
        wpair_p = ex.enter_context(tc.tile_pool(name="wpair", bufs=2))
        qk_p = ex.enter_context(tc.tile_pool(name="qk", bufs=2))
        vq_p = ex.enter_context(tc.tile_pool(name="vq", bufs=2))
        wv_p = ex.enter_context(tc.tile_pool(name="wv", bufs=2))
        exp_p = ex.enter_context(tc.tile_pool(name="expp", bufs=6))
        rz_p = ex.enter_context(tc.tile_pool(name="rz", bufs=1))
        stg_p = ex.enter_context(tc.tile_pool(name="stg", bufs=4))
        oc_p = ex.enter_context(tc.tile_pool(name="ocl", bufs=2))
        out_p = ex.enter_context(tc.tile_pool(name="outp", bufs=2))
        # PSUM budget (8 banks of [128, 2KB]):
        #   proj 2 x [128,512]  = 2 banks
        #   sA/sB 1 x [128,1024] each = 4 banks (rz-broadcast reuses sA slot)
        #   cA/cB 1 x [65,512] each = 2 banks
        ps_proj = ex.enter_context(tc.tile_pool(name="psp", bufs=2, space="PSUM"))
        ps_sc = ex.enter_context(tc.tile_pool(name="pssc", bufs=1, space="PSUM"))
        ps_ctx = ex.enter_context(tc.tile_pool(name="psctx", bufs=1, space="PSUM"))

        # ---- weight slice loaders (first quad/pair hoisted before xt) ----
        def load_wv(quad):
            wv_sb = wv_p.tile([128, 8, 512], BF16, tag="wv", name=f"wv{quad}")
            nc.sync.dma_start(
                out=wv_sb[:],
                in_=wvt_d.ap()[:, bass.ts(quad, 512)].rearrange(
                    "(dt p) e -> p dt e", p=128))
            return wv_sb

        def load_wqk(hp):
            wq_sb = wpair_p.tile([128, 8, 128], BF16, tag="wq", name=f"wq{hp}")
            nc.sync.dma_start(
                out=wq_sb[:],
                in_=wqt_d.ap()[:, bass.ts(hp, 128)].rearrange(
                    "(dt p) e -> p dt e", p=128))
            wk_sb = wpair_p.tile([128, 8, 128], BF16, tag="wk", name=f"wk{hp}")
            nc.sync.dma_start(
                out=wk_sb[:],
                in_=wkt_d.ap()[:, bass.ts(hp, 128)].rearrange(
                    "(dt p) e -> p dt e", p=128))
            return wq_sb, wk_sb

        pre_wv = load_wv(0)
        pre_wqk = load_wqk(0)

        # ---- constants / big resident tensors ----
        xt_sb = const_p.tile([128, 8, S], BF16)  # [d%128, d//128, s]
        # split the 8MB load so the first consumers start after ~2MB
        xt_re = xt_d.ap().rearrange("(dt p) s -> p dt s", p=128)
        for sc in range(4):
            nc.sync.dma_start(out=xt_sb[:, :, bass.ts(sc, 512)],
                              in_=xt_re[:, :, bass.ts(sc, 512)])
        cc_sb = const_p.tile([128, S], BF16)
        nc.sync.dma_start(out=cc_sb[:], in_=cc_d.ap())
        sg_sb = const_p.tile([128, S], BF16)
        nc.sync.dma_start(out=sg_sb[:], in_=sg_d.ap())
        bq_sb = const_p.tile([1, D], BF16)
        nc.sync.dma_start(out=bq_sb[:], in_=bq_d.ap())
        bk_sb = const_p.tile([1, D], BF16)
        nc.sync.dma_start(out=bk_sb[:], in_=bk_d.ap())
        ones_bf = const_p.tile([1, 512], BF16)
        nc.vector.memset(ones_bf[:], 1.0)
        ones_f = const_p.tile([128, HD], F32)
        nc.vector.memset(ones_f[:], 1.0)
        ones_rz = const_p.tile([128, HD], F32R)
        nc.vector.tensor_copy(ones_rz[:], ones_f[:])

        # Pin the ACT table set that serves BOTH Exp and Ln so the
        # table-load pass doesn't thrash between exp_and_others and
        # natural_log (2.7us per switch, 64 switches otherwise).
        nc.scalar.add_instruction(mybir.InstLoadActFuncSet(
            name=nc.get_next_instruction_name(),
            act_func_set_id=6,  # natural_log_exp_and_others (gen3)
            ins=[], outs=[]))

        def proj_qk(w_sb, b_sb, hp, n_chunks, dst):
            """dst[e',s-chunks] = (x @ W^T)^T + b, e' rows of pair hp."""
            for ch in range(n_chunks):
                p_ps = ps_proj.tile([128, 512], F32, tag="proj")
                for dt in range(8):
                    nc.tensor.matmul(p_ps[:], w_sb[:, dt, :],
                                     xt_sb[:, dt, bass.ts(ch, 512)],
                                     start=(dt == 0),
                                     stop=(not with_bias and dt == 7))
                if with_bias:
                    nc.tensor.matmul(p_ps[:], b_sb[0:1, bass.ts(hp, 128)],
                                     ones_bf[0:1, :], start=False, stop=True)
                nc.vector.tensor_copy(dst[:, bass.ts(ch, 512)], p_ps[:])

        def rope(raw, sw, ncols):
            """in-place: raw <- rot(raw), using sw as scratch."""
            nc.vector.stream_shuffle(sw[:, 0:ncols], raw[:, 0:ncols], _EO_MASK)
            nc.vector.tensor_mul(sw[:, 0:ncols], sw[:, 0:ncols],
                                 sg_sb[:, 0:ncols])
            nc.vector.tensor_mul(raw[:, 0:ncols], raw[:, 0:ncols],
                                 cc_sb[:, 0:ncols])
            nc.vector.tensor_add(raw[:, 0:ncols], raw[:, 0:ncols],
                                 sw[:, 0:ncols])

        for quad in range(2):
            # ---- V for 8 heads (e columns quad*512 ...) ----
            wv_sb = pre_wv if quad == 0 else load_wv(quad)
            # v_sb[kt][pq][0:64]=headA, col 64=ones, cols 66:130=headB, col 130=ones
            v_sb = vq_p.tile([128, 16, 4, 131], BF16, tag="vsb")
            nc.gpsimd.memset(v_sb[:, :, :, 64:66], 1.0)
            nc.gpsimd.memset(v_sb[:, :, :, 130:131], 1.0)
            for st in range(16):
                v_ps = ps_proj.tile([128, 512], F32, tag="proj")
                for dt in range(8):
                    nc.tensor.matmul(v_ps[:], xt_sb[:, dt, bass.ts(st, 128)],
                                     wv_sb[:, dt, :],
                                     start=(dt == 0), stop=(dt == 7))
                for pq in range(4):
                    vdst = v_sb[:, st, pq, :]
                    dst_ap = bass.AP(tensor=vdst.tensor, offset=vdst.offset,
                                     ap=[vdst.ap[0], [66, 2], [1, 64]])
                    nc.vector.tensor_copy(
                        dst_ap,
                        v_ps[:, bass.ts(pq, 128)].rearrange(
                            "p (j e) -> p j e", j=2))

            for pq in range(4):
                hp = quad * 4 + pq
                # ---- Q^T / K^T projections + rope ----
                wq_sb, wk_sb = pre_wqk if hp == 0 else load_wqk(hp)
                qt = qk_p.tile([128, SQ], BF16, tag="qt")
                kt_t = qk_p.tile([128, S], BF16, tag="kt")
                sw = qk_p.tile([128, S], BF16, tag="sw")
                proj_qk(wq_sb, bq_sb, hp, 2, qt)
                proj_qk(wk_sb, bk_sb, hp, 4, kt_t)
                rope(qt, sw, SQ)
                rope(kt_t, sw, S)

                # ---- attention, 2 heads, q in 2 chunks of 512 ----
                for qc in range(2):
                    cA = ps_ctx.tile([65, 512], F32, tag="cA")
                    cB = ps_ctx.tile([65, 512], F32, tag="cB")
                    for kt2 in range(8):
                        sA = ps_sc.tile([128, 1024], F32, tag="sA")
                        sB = ps_sc.tile([128, 1024], F32, tag="sB")
                        for j in range(2):
                            kt = kt2 * 2 + j
                            nc.tensor.matmul(
                                sA[:, bass.ts(j, 512)],
                                kt_t[0:64, bass.ts(kt, 128)],
                                qt[0:64, bass.ts(qc, 512)],
                                start=True, stop=True)
                            nc.tensor.matmul(
                                sB[:, bass.ts(j, 512)],
                                kt_t[64:128, bass.ts(kt, 128)],
                                qt[64:128, bass.ts(qc, 512)],
                                start=True, stop=True)
                        eA = exp_p.tile([128, 1024], BF16, tag="e")
                        nc.scalar.activation(eA[:], sA[:],
                                             mybir.ActivationFunctionType.Exp,
                                             scale=0.125)
                        eB = exp_p.tile([128, 1024], BF16, tag="e")
                        nc.scalar.activation(eB[:], sB[:],
                                             mybir.ActivationFunctionType.Exp,
                                             scale=0.125)
                        for j in range(2):
                            kt = kt2 * 2 + j
                            nc.tensor.matmul(cA[:], v_sb[:, kt, pq, 0:65],
                                             eA[:, bass.ts(j, 512)],
                                             start=(kt == 0), stop=(kt == 15))
                            nc.tensor.matmul(cB[:], v_sb[:, kt, pq, 66:131],
                                             eB[:, bass.ts(j, 512)],
                                             start=(kt == 0), stop=(kt == 15))
                    # normalize + spill ctx^T
                    stg = (stg_p.tile([128, 512], BF16, tag="stg", name="stg")
                           if PACKED else None)
                    for hh, cps in ((0, cA), (1, cB)):
                        lnz = rz_p.tile([65, 512], F32, tag="lnz")
                        nc.scalar.activation(lnz[64:65, :], cps[64:65, :],
                                             mybir.ActivationFunctionType.Ln)
                        rz = rz_p.tile([65, 512], F32, tag="rzf")
                        nc.scalar.activation(rz[64:65, :], lnz[64:65, :],
                                             mybir.ActivationFunctionType.Exp,
                                             scale=-1.0)
                        rzr = rz_p.tile([65, 512], F32R, tag="rzr")
                        nc.vector.tensor_copy(rzr[64:65, :], rz[64:65, :])
                        rb = ps_sc.tile([128, 1024], F32, tag="sA")
                        nc.tensor.matmul(rb[0:64, 0:512],
                                         ones_rz[64:65, 0:64],
                                         rzr[64:65, :],
                                         start=True, stop=True)
                        rbs = rz_p.tile([HD, 512], F32, tag="rbs")
                        nc.vector.tensor_copy(rbs[:], rb[0:64, 0:512])
                        if PACKED:
                            nc.vector.tensor_mul(
                                stg[bass.ts(hh, HD), :], cps[0:64, :], rbs[:])
                        else:
                            stg2 = stg_p.tile([HD, 512], BF16, tag="stg")
                            nc.vector.tensor_mul(stg2[:], cps[0:64, :], rbs[:])
                            h = 2 * hp + hh
                            nc.sync.dma_start(
                                out=ctx_d.ap()[h, :, bass.ts(qc, 512)],
                                in_=stg2[:])
                    if PACKED:
                        nc.sync.dma_start(
                            out=ctx_d.ap()[hp, :, bass.ts(qc, 512)],
                            in_=stg[:])

        # ---- out-projection: out[s, e] = sum_h ctxT_h.T @ WoT_h ----
        if PACKED:
            wot_sb = const_p.tile([128, 8, D], BF16)
            nc.sync.dma_start(out=wot_sb[:],
                              in_=wot_d.ap().rearrange("g p e -> p g e"))
        else:
            wot_sb = const_p.tile([HD, H, D], BF16)
            nc.sync.dma_start(out=wot_sb[:],
                              in_=wot_d.ap().rearrange("h p e -> p h e"))
        ng = 8 if PACKED else H  # contraction groups (pairs K=128 or heads K=64)
        for st in range(8):
            octx = oc_p.tile([128 if PACKED else HD, ng, 128], BF16, tag="oc")
            nc.sync.dma_start(
                out=octx[:],
                in_=ctx_d.ap()[:, :, bass.ts(st, 128)].rearrange(
                    "h p s -> p h s"))
            o_sb = out_p.tile([128, D], F32, tag="ot")
            for ec in range(2):
                o_ps = ps_proj.tile([128, 512], F32, tag="proj")
                for g in range(ng):
                    nc.tensor.matmul(o_ps[:], octx[:, g, :],
                                     wot_sb[:, g, bass.ts(ec, 512)],
                                     start=(g == 0), stop=(g == ng - 1))
                nc.vector.tensor_copy(o_sb[:, bass.ts(ec, 512)], o_ps[:])
            nc.sync.dma_start(out=out_d.ap()[bass.ts(st, 128), :], in_=o_sb[:])

    nc.finalize()
    return nc


_NC = {}


def _get_nc(with_bias=True):
    if with_bias not in _NC:
        _NC[with_bias] = _build_kernel(with_bias)
    return _NC[with_bias]


def _host_prep(hidden_states, Wq, bq, Wk, bk, Wv, bv, Wo, bo):
    """Build per-core input maps (host does layout transforms only)."""
    f32 = np.float32
    hidden_states = np.asarray(hidden_states, f32)
    Wq, Wk, Wv, Wo = (np.asarray(w, f32) for w in (Wq, Wk, Wv, Wo))
    bq, bk, bv, bo = (np.asarray(b, f32) for b in (bq, bk, bv, bo))

    # interleave permutation: new row 64*blk + 2*i + t <- old row 64*blk+32*t+i
    p = np.arange(D)
    blk, r = p // HD, p % HD
    perm = blk * HD + (r % 2) * 32 + (r // 2)

    wqt = np.ascontiguousarray(Wq[perm].T).astype(NP_BF16)
    wkt = np.ascontiguousarray(Wk[perm].T).astype(NP_BF16)
    wvt = np.ascontiguousarray(Wv.T).astype(NP_BF16)
    if PACKED:
        wot = np.ascontiguousarray(Wo.T).reshape(8, 128, D).astype(NP_BF16)
    else:
        wot = np.ascontiguousarray(Wo.T).reshape(H, HD, D).astype(NP_BF16)
    bq_i = bq[perm].reshape(1, D).astype(NP_BF16)
    bk_i = bk[perm].reshape(1, D).astype(NP_BF16)

    # rope tables (reference quirk: "c" is sin, "s" is cos), interleaved rows
    inv_freq = 1.0 / (10000.0 ** (np.arange(0, HD, 2, dtype=f32) / HD))
    ang = np.arange(S, dtype=f32)[:, None] * inv_freq[None, :]  # [S, 32]
    sin_t, cos_t = np.sin(ang), np.cos(ang)  # float64? no: f32 in, f64 out
    rows = np.arange(128)
    i_of = (rows % HD) // 2
    sign = np.where(rows % 2 == 0, -1.0, 1.0)
    cc = sin_t.T[i_of, :].astype(NP_BF16)                      # [128, S]
    sg = (cos_t.T[i_of, :] * sign[:, None]).astype(NP_BF16)    # [128, S]

    in_maps = []
    for c in range(NCORE):
        b_i, qh = c // 2, c % 2
        own = slice(qh * SQ, (qh + 1) * SQ)
        other = slice((1 - qh) * SQ, (2 - qh) * SQ)
        col = np.r_[np.arange(qh * SQ, (qh + 1) * SQ),
                    np.arange((1 - qh) * SQ, (2 - qh) * SQ)]
        xt = np.ascontiguousarray(hidden_states[b_i].T[:, col]).astype(NP_BF16)
        in_maps.append({
            "xt": xt,
            "wqt": wqt, "wkt": wkt, "wvt": wvt, "wot": wot,
            "bq": bq_i, "bk": bk_i,
            "cc": np.ascontiguousarray(cc[:, col]),
            "sg": np.ascontiguousarray(sg[:, col]),
        })
    # host-folded output constant: sum_dd Wo[e,dd]*bv[dd] + bo[e]
    out_const = (Wo @ bv + bo).astype(f32)
    return in_maps, out_const


def kernel(hidden_states, Wq, bq, Wk, bk, Wv, bv, Wo, bo, _trace=False):
    in_maps, out_const = _host_prep(hidden_states, Wq, bq, Wk, bk, Wv, bv,
                                    Wo, bo)
    with_bias = bool(np.any(np.asarray(bq)) or np.any(np.asarray(bk)))
    nc = _get_nc(with_bias)
    res = run_bass_kernel_spmd(nc, in_maps, core_ids=list(range(NCORE)),
                               trace=_trace)
    out = np.empty((B, S, D), np.float32)
    for c in range(NCORE):
        b_i, qh = c // 2, c % 2
        out[b_i, qh * SQ:(qh + 1) * SQ, :] = res.results[c]["out"]
    out += out_const[None, None, :]
    if _trace:
        return out, res
    return out
